# revision 9
# baseline (speedup 1.0000x reference)
"""Causal self-attention (SEQ=8192, D=1024) on 8 TRN2 NeuronCores.

Strategy (SPMD, one static graph on all 8 cores, ZERO collectives):
  - scores = x_q (Wq^T Wk) x_k^T: fold the combined weight M = Wq^T Wk
    into the QUERY side. Each core projects only its own 1024 strided
    queries (Q''^T = M^T x_q^T, ~27us of PE); the keys are the RAW
    input x^T, replicated to every core's HBM by the host and streamed
    from local DRAM. No K AllGather.
  - O = (P @ x) Wv^T: apply Wv on the OUTPUT side. PV accumulates
    O1 = P^T.T @ x_chunk (raw x as "values", again local HBM), and a
    final per-core [1024q x 1024] @ Wv^T projection replaces the
    sharded V projection at identical FLOP cost. No V AllGather.
  - Sequence-parallel over queries with stride-8 row interleaving
    (core i owns query rows {8j+i}) exactly balances causal work while
    keeping one SPMD graph; per-core differences are data only
    (x_q^T slice + causal masks).
  - Attention runs in S^T layout over 16 key blocks of 512: S^T chunk =
    x_k-chunk^T.T @ Q''^T, exp on ScalarE (scale fused), diagonal-block
    masking by a data mask, denominator via a ones column baked into
    the streamed x (ones-matmul accumulated alongside O1 in PSUM).
  - Output: O1 (f32) -> bf16 -> XBAR DMA-transpose (free, on DMA
    engines) -> 16 matmuls vs Wv^T chunks -> scale by 1/denominator.
    The out-projection for J is deferred by two key blocks so the
    add/cast/transpose latency never blocks the Tensor queue; J=6's
    runs after PV(15) to keep the PE warm through J=7's finalize.
  - All inputs are host-pre-arranged so every DMA is a [128, N]
    contiguous transfer (128 descriptors, no descriptor storms).
  - All matmul operands bf16 (1 cyc/row on the PE), accumulation fp32.
"""
import sys

sys.path.insert(0, "/opt/trn_rl_repo")

import numpy as np
import ml_dtypes

import concourse.bacc as bacc
import concourse.mybir as mybir
import concourse.tile as tile
from concourse import bass_utils

S, D, NC = 8192, 1024, 8
QPC = S // NC  # 1024 queries per core
NCH = D // 128  # 8 chunks of the feature dim
NQT = QPC // 128  # 8 query tiles (J) per core
NKB = S // 512  # 16 key blocks of 512
XFW = 1028  # xf row width: 1024 feats + ones col + pad
SCALE = 1.0 / np.sqrt(D).astype(np.float32)  # 1/32
BF16 = mybir.dt.bfloat16
F32 = mybir.dt.float32

_cache = {}


def _build():
    if "nc" in _cache:
        return _cache["nc"]
    nc = bacc.Bacc("TRN2", target_bir_lowering=False, debug=False, num_devices=NC)

    # all pre-arranged on host: partition dim first, contiguous free dim
    m_in = nc.dram_tensor("m", [128, NCH * D], BF16, kind="ExternalInput")
    xq0_in = nc.dram_tensor("xq0", [128, NCH * 512], BF16, kind="ExternalInput")
    xq1_in = nc.dram_tensor("xq1", [128, NCH * 512], BF16, kind="ExternalInput")
    xt_in = nc.dram_tensor("xt", [NKB, 128, NCH * 512], BF16, kind="ExternalInput")
    xf_in = nc.dram_tensor("xf", [NKB, 128, 4 * XFW], BF16, kind="ExternalInput")
    wv_in = nc.dram_tensor("wv", [128, NCH * D], BF16, kind="ExternalInput")
    mask_in = nc.dram_tensor("masks", [128, 8 * 128], BF16, kind="ExternalInput")
    out = nc.dram_tensor("out", [QPC, D], F32, kind="ExternalOutput")

    with tile.TileContext(nc) as tc:
        with (
            tc.tile_pool(name="persist", bufs=1) as persist,
            tc.tile_pool(name="kv", bufs=3) as kv,
            tc.tile_pool(name="kvv", bufs=3) as kvv,
            tc.tile_pool(name="ptp", bufs=2) as ptp,
            tc.tile_pool(name="fin", bufs=2) as fin,
        ):
            sb_qt = persist.tile([128, NCH * QPC], BF16, tag="qt")
            sb_wv = persist.tile([128, NCH * D], BF16, tag="wv")
            sb_mask = persist.tile([128, 8 * 128], BF16, tag="msk")
            o_acc = [
                persist.tile([128, D + 1], F32, tag=f"oacc{j}", name=f"oacc{j}")
                for j in range(NQT)
            ]

            # ---- Q'' projection: sb_qt = (M^T x_q^T) chunks ----
            # m/xq loaded as per-chunk tiles on two DMA queues so the first
            # chain starts after ~0.5MB instead of after the full 4MB.
            with (
                tc.tile_pool(name="io", bufs=1) as io,
                tc.tile_pool(name="pp", bufs=4, space="PSUM") as pp,
            ):
                sb_m = [
                    io.tile([128, D], BF16, tag=f"m{c}", name=f"m{c}")
                    for c in range(NCH)
                ]
                sb_xq = [
                    [
                        io.tile([128, 512], BF16, tag=f"xq{h}_{c}", name=f"xq{h}_{c}")
                        for c in range(NCH)
                    ]
                    for h in range(2)
                ]
                for c in range(NCH):
                    nc.sync.dma_start(sb_m[c][:], m_in[:, c * D : (c + 1) * D])
                    nc.scalar.dma_start(
                        sb_xq[0][c][:], xq0_in[:, c * 512 : (c + 1) * 512]
                    )
                for c in range(NCH):
                    nc.scalar.dma_start(
                        sb_xq[1][c][:], xq1_in[:, c * 512 : (c + 1) * 512]
                    )

                for half in range(2):
                    for fo in range(NCH):
                        ps = pp.tile([128, 512], F32, tag="pp", name="ps")
                        for c in range(NCH):
                            nc.tensor.matmul(
                                ps[:],
                                sb_m[c][:, fo * 128 : fo * 128 + 128],
                                sb_xq[half][c][:],
                                start=(c == 0),
                                stop=(c == NCH - 1),
                            )
                        nc.any.tensor_copy(
                            sb_qt[
                                :,
                                fo * QPC + half * 512 : fo * QPC + half * 512 + 512,
                            ],
                            ps[:],
                        )

            # ---- attention over 16 key blocks of 512 ----
            with (
                tc.tile_pool(name="psst", bufs=2, space="PSUM") as psst,
                tc.tile_pool(name="pso", bufs=2, space="PSUM") as pso,
            ):
                nc.sync.dma_start(sb_mask[:], mask_in[:])
                nc.sync.dma_start(sb_wv[:], wv_in[:])

                def j_groups(Sb):
                    if Sb + 4 < NQT:
                        return [(Sb, Sb + 4), (Sb + 4, NQT)]
                    return [(Sb, NQT)]

                def st_block(kb):
                    Sb, H = kb >> 1, kb & 1
                    kt_t = kv.tile([128, NCH * 512], BF16, tag="kt", name="kt_t")
                    nc.sync.dma_start(kt_t[:], xt_in[kb])
                    pts = {}
                    for kt4 in range(4):
                        kt = H * 4 + kt4
                        for gi, (j0, j1) in enumerate(j_groups(Sb)):
                            N = (j1 - j0) * 128
                            q0 = max(0, 16 * kt - 1) if j0 == Sb else 0
                            st = psst.tile([128, 512], F32, tag="st", name="st")
                            for c in range(NCH):
                                nc.tensor.matmul(
                                    st[:, q0:N],
                                    kt_t[
                                        :,
                                        c * 512 + kt4 * 128 : c * 512 + kt4 * 128 + 128,
                                    ],
                                    sb_qt[
                                        :,
                                        c * QPC + j0 * 128 + q0 : c * QPC + j1 * 128,
                                    ],
                                    start=(c == 0),
                                    stop=(c == NCH - 1),
                                )
                            pt = ptp.tile(
                                [128, 512],
                                BF16,
                                tag=f"pt{gi}_{kt4}",
                                name=f"pt{gi}_{kt4}",
                            )
                            nc.scalar.activation(
                                pt[:, 0:N],
                                st[:, 0:N],
                                mybir.ActivationFunctionType.Exp,
                                scale=float(SCALE),
                            )
                            if j0 == Sb:
                                nc.vector.tensor_mul(
                                    pt[:, 0:128],
                                    pt[:, 0:128],
                                    sb_mask[:, kt * 128 : kt * 128 + 128],
                                )
                            pts[(gi, kt4)] = pt
                    return pts

                def finalize_front(J, o_ps):
                    """Last PV chunk for J: fuse the final o_acc add with the
                    bf16 cast (split in halves so the first XBAR transpose
                    starts early), then reciprocal. PE-side out-projection is
                    deferred (outproj_part)."""
                    o1 = fin.tile([128, D], BF16, tag="o1", name="o1")
                    o1t = fin.tile([128, NCH * 128], BF16, tag="o1t", name="o1t")
                    o1t3 = o1t.rearrange("p (c q) -> p c q", c=NCH)
                    for h in range(2):
                        nc.vector.tensor_add(
                            o1[:, h * 512 : h * 512 + 512],
                            o_acc[J][:, h * 512 : h * 512 + 512],
                            o_ps[:, h * 512 : h * 512 + 512],
                        )
                        nc.scalar.dma_start_transpose(
                            o1t3[:, h * 4 : h * 4 + 4, :],
                            o1[:, h * 512 : h * 512 + 512],
                        )
                    dd = fin.tile([128, 1], F32, tag="dd", name="dd")
                    nc.vector.tensor_add(
                        dd[:], o_acc[J][:, D : D + 1], o_ps[:, D : D + 1]
                    )
                    rec = fin.tile([128, 1], F32, tag="rec", name="rec")
                    nc.vector.reciprocal(rec[:], dd[:])
                    return rec, o1t

                def outproj_part(J, rec, o1t, part, out_ps):
                    """Half the contraction (chunks 4*part..4*part+4) for both
                    output halves; PSUM accumulation pauses between parts."""
                    o1t3 = o1t.rearrange("p (c q) -> p c q", c=NCH)
                    for half in range(2):
                        for cc in range(4):
                            c = part * 4 + cc
                            nc.tensor.matmul(
                                out_ps[:, half * 512 : half * 512 + 512],
                                o1t3[:, c, :],
                                sb_wv[
                                    :,
                                    c * D + half * 512 : c * D + half * 512 + 512,
                                ],
                                start=(c == 0),
                                stop=(c == NCH - 1),
                            )
                    if part == 1:
                        outt = fin.tile([128, D], F32, tag="outt", name="outt")
                        nc.scalar.activation(
                            outt[:],
                            out_ps[:, 0:D],
                            mybir.ActivationFunctionType.Copy,
                            scale=rec[:],
                        )
                        nc.gpsimd.dma_start(out[J * 128 : (J + 1) * 128, :], outt[:])

                def outproj_pe(J, rec, o1t):
                    out_ps = pso.tile([128, 1536], F32, tag="ops", name="out_ps")
                    outproj_part(J, rec, o1t, 0, out_ps)
                    outproj_part(J, rec, o1t, 1, out_ps)

                def pv_block(kb, pts):
                    Sb = kb >> 1
                    v_t = kvv.tile([128, 4 * XFW], BF16, tag="v", name="v_t")
                    nc.sync.dma_start(v_t[:], xf_in[kb])
                    done = None
                    for J in range(Sb, NQT):
                        gi = 0 if J < min(Sb + 4, NQT) else 1
                        j0 = Sb if gi == 0 else Sb + 4
                        o_ps = pso.tile([128, 1536], F32, tag="ops", name="o_ps")
                        for kt4 in range(4):
                            pt = pts[(gi, kt4)]
                            lhsT = pt[:, (J - j0) * 128 : (J - j0 + 1) * 128]
                            for dh in range(2):
                                nc.tensor.matmul(
                                    o_ps[:, dh * 512 : dh * 512 + 512],
                                    lhsT,
                                    v_t[
                                        :,
                                        kt4 * XFW
                                        + dh * 512 : kt4 * XFW
                                        + dh * 512
                                        + 512,
                                    ],
                                    start=(kt4 == 0),
                                    stop=(kt4 == 3),
                                )
                            nc.tensor.matmul(
                                o_ps[:, 1024:1025],
                                lhsT,
                                v_t[:, kt4 * XFW + 1024 : kt4 * XFW + 1025],
                                start=(kt4 == 0),
                                stop=(kt4 == 3),
                            )
                        if kb == 2 * J + 1:
                            done = (J, *finalize_front(J, o_ps))
                        elif kb == 0:
                            nc.vector.tensor_copy(o_acc[J][:], o_ps[:, 0 : D + 1])
                        else:
                            nc.vector.tensor_add(
                                o_acc[J][:], o_acc[J][:], o_ps[:, 0 : D + 1]
                            )
                    return done

                # outproj(J) runs after pv_block(2J+3): >= one full ST+PV of
                # cover for the add/cast/transpose chain, and J=6 lands after
                # PV(15) keeping the PE warm through J=7's finalize.
                ready = {}
                for kb in range(NKB):
                    pts = st_block(kb)
                    done = pv_block(kb, pts)
                    if done is not None:
                        ready[done[0]] = done[1:]
                    J_out = (kb - 3) // 2
                    if kb >= 3 and kb % 2 == 1 and J_out in ready:
                        outproj_pe(J_out, *ready.pop(J_out))
                for J in sorted(ready):
                    outproj_pe(J, *ready.pop(J))

    nc.compile()
    _cache["nc"] = nc
    return nc


def _make_in_maps(inputs, w_query, w_key, w_value):
    bf = ml_dtypes.bfloat16
    x32 = inputs.astype(np.float32)
    xb = x32.astype(bf)
    xt = np.ascontiguousarray(xb.T)  # [D, S]
    # [16, 128, 8c*512] : xt_pre[kb, p, c*512+k] = x[kb*512+k, c*128+p]
    xt_pre = np.ascontiguousarray(
        xt.reshape(NCH, 128, NKB, 512).transpose(2, 1, 0, 3).reshape(NKB, 128, -1)
    )
    xf = np.zeros((S, XFW), dtype=bf)
    xf[:, 0:D] = xb
    xf[:, D] = np.float32(1.0)
    # [16, 128, 4c*1028] : xf_pre[kb, p, c*1028+w] = xf[kb*512+c*128+p, w]
    xf_pre = np.ascontiguousarray(
        xf.reshape(NKB, 4, 128, XFW).transpose(0, 2, 1, 3).reshape(NKB, 128, -1)
    )

    def fold(a):  # [1024, W] -> [128, 8*W] with chunk c at cols [c*W, (c+1)*W)
        W = a.shape[1]
        return np.ascontiguousarray(
            a.reshape(NCH, 128, W).transpose(1, 0, 2).reshape(128, NCH * W)
        )

    m = (w_query.astype(np.float32).T @ w_key.astype(np.float32)).astype(bf)
    m_pre = fold(m)
    wv_pre = fold(np.ascontiguousarray(w_value.T).astype(bf))

    kt_off = np.arange(8)[:, None, None] * 128 + np.arange(128)[None, :, None]
    in_maps = []
    for i in range(NC):
        xq = np.ascontiguousarray(xt[:, i::NC])  # [D, QPC]
        xq0 = fold(xq[:, 0:512])
        xq1 = fold(xq[:, 512:1024])
        q_off = np.arange(128)[None, None, :] * 8 + i
        mask = (kt_off <= q_off).astype(np.float32).astype(bf)  # [8,128,128]
        mask_pre = np.ascontiguousarray(
            mask.transpose(1, 0, 2).reshape(128, 8 * 128)
        )
        in_maps.append(
            {
                "m": m_pre,
                "xq0": xq0,
                "xq1": xq1,
                "xt": xt_pre,
                "xf": xf_pre,
                "wv": wv_pre,
                "masks": mask_pre,
            }
        )
    return in_maps


def run(inputs, w_query, w_key, w_value, trace=False):
    nc = _build()
    in_maps = _make_in_maps(inputs, w_query, w_key, w_value)
    res = bass_utils.run_bass_kernel_spmd(
        nc, in_maps, core_ids=list(range(NC)), trace=trace
    )
    full = np.empty((S, D), dtype=np.float32)
    for i in range(NC):
        full[i::NC] = res.results[i]["out"]
    return full, res


def kernel(inputs, w_query, w_key, w_value):
    inputs = np.asarray(inputs, dtype=np.float32)
    w_query = np.asarray(w_query, dtype=np.float32)
    w_key = np.asarray(w_key, dtype=np.float32)
    w_value = np.asarray(w_value, dtype=np.float32)
    full, _ = run(inputs, w_query, w_key, w_value, trace=False)
    return full


# revision 14
# speedup vs baseline: 1.1853x; 1.1853x over previous
"""Causal self-attention (SEQ=8192, D=1024) on 8 TRN2 NeuronCores.

Strategy (SPMD, one static graph on all 8 cores, ZERO collectives):
  - scores = x_q (Wq^T Wk) x_k^T: fold the combined weight M = Wq^T Wk
    into the QUERY side. Each core projects only its own 1024 strided
    queries (Q''^T = M^T x_q^T, ~27us of PE); the keys are the RAW
    input x^T, replicated to every core's HBM by the host and streamed
    from local DRAM. No K AllGather.
  - O = (P @ x) Wv^T: apply Wv on the OUTPUT side. PV accumulates
    O1 = P^T.T @ x_chunk (raw x as "values", again local HBM), and a
    final per-core [1024q x 1024] @ Wv^T projection replaces the
    sharded V projection at identical FLOP cost. No V AllGather.
  - Sequence-parallel over queries with stride-8 row interleaving
    (core i owns query rows {8j+i}) exactly balances causal work while
    keeping one SPMD graph; per-core differences are data only
    (x_q^T slice + causal masks).
  - Attention runs in S^T layout over 16 key blocks of 512: S^T chunk =
    x_k-chunk^T.T @ Q''^T, exp on ScalarE (scale fused), diagonal-block
    masking by a data mask, denominator via a ones column baked into
    the streamed x (ones-matmul accumulated alongside O1 in PSUM).
  - Output: O1 (f32) -> bf16 -> XBAR DMA-transpose (free, on DMA
    engines) -> 16 matmuls vs Wv^T chunks -> scale by 1/denominator.
    The out-projection for J is deferred by two key blocks so the
    add/cast/transpose latency never blocks the Tensor queue; J=6's
    runs after PV(15) to keep the PE warm through J=7's finalize.
  - All inputs are host-pre-arranged so every DMA is a [128, N]
    contiguous transfer (128 descriptors, no descriptor storms).
  - All matmul operands bf16 (1 cyc/row on the PE), accumulation fp32.
"""
import sys

sys.path.insert(0, "/opt/trn_rl_repo")

import numpy as np
import ml_dtypes

import concourse.bacc as bacc
import concourse.mybir as mybir
import concourse.tile as tile
from concourse import bass_utils

S, D, NC = 8192, 1024, 8
QPC = S // NC  # 1024 queries per core
NCH = D // 128  # 8 chunks of the feature dim
NQT = QPC // 128  # 8 query tiles (J) per core
NKB = S // 512  # 16 key blocks of 512
XFW = 1028  # xf row width: 1024 feats + ones col + pad
SCALE = 1.0 / np.sqrt(D).astype(np.float32)  # 1/32
BF16 = mybir.dt.bfloat16
F32 = mybir.dt.float32

_cache = {}


def _build():
    if "nc" in _cache:
        return _cache["nc"]
    nc = bacc.Bacc("TRN2", target_bir_lowering=False, debug=False, num_devices=NC)

    # all pre-arranged on host: partition dim first, contiguous free dim
    m_in = nc.dram_tensor("m", [128, NCH * D], BF16, kind="ExternalInput")
    xq0_in = nc.dram_tensor("xq0", [128, NCH * 512], BF16, kind="ExternalInput")
    xq1_in = nc.dram_tensor("xq1", [128, NCH * 512], BF16, kind="ExternalInput")
    xt_in = nc.dram_tensor("xt", [NKB, 128, NCH * 512], BF16, kind="ExternalInput")
    xf_in = nc.dram_tensor("xf", [NKB, 128, 4 * XFW], BF16, kind="ExternalInput")
    wv_in = nc.dram_tensor("wv", [128, NCH * D], BF16, kind="ExternalInput")
    mask_in = nc.dram_tensor("masks", [128, 8 * 128], BF16, kind="ExternalInput")
    out = nc.dram_tensor("out", [QPC, D], F32, kind="ExternalOutput")

    with tile.TileContext(nc) as tc:
        with (
            tc.tile_pool(name="persist", bufs=1) as persist,
            tc.tile_pool(name="kv", bufs=3) as kv,
            tc.tile_pool(name="kvv", bufs=3) as kvv,
            tc.tile_pool(name="ptp", bufs=2) as ptp,
            tc.tile_pool(name="fin", bufs=2) as fin,
        ):
            sb_qt = persist.tile([128, NCH * QPC], BF16, tag="qt")
            sb_wv = persist.tile([128, NCH * D], BF16, tag="wv")
            sb_mask = persist.tile([128, 8 * 128], BF16, tag="msk")
            o_acc = [
                persist.tile([128, D + 1], F32, tag=f"oacc{j}", name=f"oacc{j}")
                for j in range(NQT)
            ]

            # ---- Q'' projection: sb_qt = (M^T x_q^T) chunks ----
            # m/xq loaded as per-chunk tiles on two DMA queues so the first
            # chain starts after ~0.5MB instead of after the full 4MB.
            with (
                tc.tile_pool(name="io", bufs=1) as io,
                tc.tile_pool(name="pp", bufs=4, space="PSUM") as pp,
            ):
                sb_m = [
                    io.tile([128, D], BF16, tag=f"m{c}", name=f"m{c}")
                    for c in range(NCH)
                ]
                sb_xq = [
                    [
                        io.tile([128, 512], BF16, tag=f"xq{h}_{c}", name=f"xq{h}_{c}")
                        for c in range(NCH)
                    ]
                    for h in range(2)
                ]
                for c in range(NCH):
                    nc.sync.dma_start(sb_m[c][:], m_in[:, c * D : (c + 1) * D])
                    nc.scalar.dma_start(
                        sb_xq[0][c][:], xq0_in[:, c * 512 : (c + 1) * 512]
                    )
                for c in range(NCH):
                    nc.scalar.dma_start(
                        sb_xq[1][c][:], xq1_in[:, c * 512 : (c + 1) * 512]
                    )

                for half in range(2):
                    for fo in range(NCH):
                        ps = pp.tile([128, 512], F32, tag="pp", name="ps")
                        for c in range(NCH):
                            nc.tensor.matmul(
                                ps[:],
                                sb_m[c][:, fo * 128 : fo * 128 + 128],
                                sb_xq[half][c][:],
                                start=(c == 0),
                                stop=(c == NCH - 1),
                            )
                        nc.any.tensor_copy(
                            sb_qt[
                                :,
                                fo * QPC + half * 512 : fo * QPC + half * 512 + 512,
                            ],
                            ps[:],
                        )

            # ---- attention over 16 key blocks of 512 ----
            with (
                tc.tile_pool(name="psst", bufs=2, space="PSUM") as psst,
                tc.tile_pool(name="pso", bufs=2, space="PSUM") as pso,
            ):
                nc.sync.dma_start(sb_mask[:], mask_in[:])
                nc.sync.dma_start(sb_wv[:], wv_in[:])

                def j_groups(Sb):
                    if Sb + 4 < NQT:
                        return [(Sb, Sb + 4), (Sb + 4, NQT)]
                    return [(Sb, NQT)]

                def issue_loads(kb):
                    kt_t = kv.tile([128, NCH * 512], BF16, tag="kt", name="kt_t")
                    nc.sync.dma_start(kt_t[:], xt_in[kb])
                    v_t = kvv.tile([128, 4 * XFW], BF16, tag="v", name="v_t")
                    nc.sync.dma_start(v_t[:], xf_in[kb])
                    return kt_t, v_t

                def st_block(kb, kt_t):
                    Sb, H = kb >> 1, kb & 1
                    pts = {}
                    for kt4 in range(4):
                        kt = H * 4 + kt4
                        for gi, (j0, j1) in enumerate(j_groups(Sb)):
                            N = (j1 - j0) * 128
                            q0 = max(0, 16 * kt - 1) if j0 == Sb else 0
                            st = psst.tile([128, 512], F32, tag="st", name="st")
                            for c in range(NCH):
                                nc.tensor.matmul(
                                    st[:, q0:N],
                                    kt_t[
                                        :,
                                        c * 512 + kt4 * 128 : c * 512 + kt4 * 128 + 128,
                                    ],
                                    sb_qt[
                                        :,
                                        c * QPC + j0 * 128 + q0 : c * QPC + j1 * 128,
                                    ],
                                    start=(c == 0),
                                    stop=(c == NCH - 1),
                                )
                            pt = ptp.tile(
                                [128, 512],
                                BF16,
                                tag=f"pt{gi}_{kt4}",
                                name=f"pt{gi}_{kt4}",
                            )
                            nc.scalar.activation(
                                pt[:, 0:N],
                                st[:, 0:N],
                                mybir.ActivationFunctionType.Exp,
                                scale=float(SCALE),
                            )
                            if j0 == Sb:
                                nc.vector.tensor_mul(
                                    pt[:, 0:128],
                                    pt[:, 0:128],
                                    sb_mask[:, kt * 128 : kt * 128 + 128],
                                )
                            pts[(gi, kt4)] = pt
                    return pts

                def finalize_front(J, o_ps):
                    """Last PV chunk for J: fuse the final o_acc add with the
                    bf16 cast (split in halves so the first XBAR transpose
                    starts early), then reciprocal. PE-side out-projection is
                    deferred (outproj_part)."""
                    o1 = fin.tile([128, D], BF16, tag="o1", name="o1")
                    o1t = fin.tile([128, NCH * 128], BF16, tag="o1t", name="o1t")
                    o1t3 = o1t.rearrange("p (c q) -> p c q", c=NCH)
                    for h in range(2):
                        nc.vector.tensor_add(
                            o1[:, h * 512 : h * 512 + 512],
                            o_acc[J][:, h * 512 : h * 512 + 512],
                            o_ps[:, h * 512 : h * 512 + 512],
                        )
                        nc.sync.dma_start_transpose(
                            o1t3[:, h * 4 : h * 4 + 4, :],
                            o1[:, h * 512 : h * 512 + 512],
                        )
                    dd = fin.tile([128, 1], F32, tag="dd", name="dd")
                    nc.vector.tensor_add(
                        dd[:], o_acc[J][:, D : D + 1], o_ps[:, D : D + 1]
                    )
                    rec = fin.tile([128, 1], F32, tag="rec", name="rec")
                    nc.vector.reciprocal(rec[:], dd[:])
                    return rec, o1t

                def outproj_part(J, rec, o1t, part, out_ps):
                    """Half the contraction (chunks 4*part..4*part+4) for both
                    output halves; PSUM accumulation pauses between parts."""
                    o1t3 = o1t.rearrange("p (c q) -> p c q", c=NCH)
                    for half in range(2):
                        for cc in range(4):
                            c = part * 4 + cc
                            nc.tensor.matmul(
                                out_ps[:, half * 512 : half * 512 + 512],
                                o1t3[:, c, :],
                                sb_wv[
                                    :,
                                    c * D + half * 512 : c * D + half * 512 + 512,
                                ],
                                start=(c == 0),
                                stop=(c == NCH - 1),
                            )
                    if part == 1:
                        outt = fin.tile([128, D], F32, tag="outt", name="outt")
                        nc.vector.tensor_scalar_mul(outt[:], out_ps[:, 0:D], rec[:])
                        nc.gpsimd.dma_start(out[J * 128 : (J + 1) * 128, :], outt[:])

                def outproj_pe(J, rec, o1t):
                    out_ps = pso.tile([128, 1536], F32, tag="ops", name="out_ps")
                    outproj_part(J, rec, o1t, 0, out_ps)
                    outproj_part(J, rec, o1t, 1, out_ps)

                def pv_block(kb, pts, v_t):
                    Sb = kb >> 1
                    done = None
                    for J in range(Sb, NQT):
                        gi = 0 if J < min(Sb + 4, NQT) else 1
                        j0 = Sb if gi == 0 else Sb + 4
                        o_ps = pso.tile([128, 1536], F32, tag="ops", name="o_ps")
                        for kt4 in range(4):
                            pt = pts[(gi, kt4)]
                            lhsT = pt[:, (J - j0) * 128 : (J - j0 + 1) * 128]
                            for dh in range(2):
                                nc.tensor.matmul(
                                    o_ps[:, dh * 512 : dh * 512 + 512],
                                    lhsT,
                                    v_t[
                                        :,
                                        kt4 * XFW
                                        + dh * 512 : kt4 * XFW
                                        + dh * 512
                                        + 512,
                                    ],
                                    start=(kt4 == 0),
                                    stop=(kt4 == 3),
                                )
                            nc.tensor.matmul(
                                o_ps[:, 1024:1025],
                                lhsT,
                                v_t[:, kt4 * XFW + 1024 : kt4 * XFW + 1025],
                                start=(kt4 == 0),
                                stop=(kt4 == 3),
                            )
                        if kb == 2 * J + 1:
                            done = (J, *finalize_front(J, o_ps))
                        elif kb == 0:
                            nc.vector.tensor_copy(o_acc[J][:], o_ps[:, 0 : D + 1])
                        else:
                            nc.vector.tensor_add(
                                o_acc[J][:], o_acc[J][:], o_ps[:, 0 : D + 1]
                            )
                    return done

                # outproj(J) runs after pv_block(2J+3): >= one full ST+PV of
                # cover for the add/cast/transpose chain, and J=6 lands after
                # PV(15) keeping the PE warm through J=7's finalize.
                ready = {}
                loads = issue_loads(0)
                for kb in range(NKB):
                    next_loads = issue_loads(kb + 1) if kb + 1 < NKB else None
                    pts = st_block(kb, loads[0])
                    done = pv_block(kb, pts, loads[1])
                    loads = next_loads
                    if done is not None:
                        ready[done[0]] = done[1:]
                    J_out = (kb - 3) // 2
                    if kb >= 3 and kb % 2 == 1 and J_out in ready:
                        outproj_pe(J_out, *ready.pop(J_out))
                for J in sorted(ready):
                    outproj_pe(J, *ready.pop(J))

    nc.compile()
    _cache["nc"] = nc
    return nc


def _make_in_maps(inputs, w_query, w_key, w_value):
    bf = ml_dtypes.bfloat16
    x32 = inputs.astype(np.float32)
    xb = x32.astype(bf)
    xt = np.ascontiguousarray(xb.T)  # [D, S]
    # [16, 128, 8c*512] : xt_pre[kb, p, c*512+k] = x[kb*512+k, c*128+p]
    xt_pre = np.ascontiguousarray(
        xt.reshape(NCH, 128, NKB, 512).transpose(2, 1, 0, 3).reshape(NKB, 128, -1)
    )
    xf = np.zeros((S, XFW), dtype=bf)
    xf[:, 0:D] = xb
    xf[:, D] = np.float32(1.0)
    # [16, 128, 4c*1028] : xf_pre[kb, p, c*1028+w] = xf[kb*512+c*128+p, w]
    xf_pre = np.ascontiguousarray(
        xf.reshape(NKB, 4, 128, XFW).transpose(0, 2, 1, 3).reshape(NKB, 128, -1)
    )

    def fold(a):  # [1024, W] -> [128, 8*W] with chunk c at cols [c*W, (c+1)*W)
        W = a.shape[1]
        return np.ascontiguousarray(
            a.reshape(NCH, 128, W).transpose(1, 0, 2).reshape(128, NCH * W)
        )

    m = (w_query.astype(np.float32).T @ w_key.astype(np.float32)).astype(bf)
    m_pre = fold(m)
    wv_pre = fold(np.ascontiguousarray(w_value.T).astype(bf))

    kt_off = np.arange(8)[:, None, None] * 128 + np.arange(128)[None, :, None]
    in_maps = []
    for i in range(NC):
        xq = np.ascontiguousarray(xt[:, i::NC])  # [D, QPC]
        xq0 = fold(xq[:, 0:512])
        xq1 = fold(xq[:, 512:1024])
        q_off = np.arange(128)[None, None, :] * 8 + i
        mask = (kt_off <= q_off).astype(np.float32).astype(bf)  # [8,128,128]
        mask_pre = np.ascontiguousarray(
            mask.transpose(1, 0, 2).reshape(128, 8 * 128)
        )
        in_maps.append(
            {
                "m": m_pre,
                "xq0": xq0,
                "xq1": xq1,
                "xt": xt_pre,
                "xf": xf_pre,
                "wv": wv_pre,
                "masks": mask_pre,
            }
        )
    return in_maps


def run(inputs, w_query, w_key, w_value, trace=False):
    nc = _build()
    in_maps = _make_in_maps(inputs, w_query, w_key, w_value)
    res = bass_utils.run_bass_kernel_spmd(
        nc, in_maps, core_ids=list(range(NC)), trace=trace
    )
    full = np.empty((S, D), dtype=np.float32)
    for i in range(NC):
        full[i::NC] = res.results[i]["out"]
    return full, res


def kernel(inputs, w_query, w_key, w_value):
    inputs = np.asarray(inputs, dtype=np.float32)
    w_query = np.asarray(w_query, dtype=np.float32)
    w_key = np.asarray(w_key, dtype=np.float32)
    w_value = np.asarray(w_value, dtype=np.float32)
    full, _ = run(inputs, w_query, w_key, w_value, trace=False)
    return full
